# revision 28
# baseline (speedup 1.0000x reference)
"""Trainium2 Bass kernel for nn_MultiHeadAttention_2250562863251.

Key algebraic insight: the reference einsum 'mbhi,nbhj->mnbh' contracts i and j
independently, so scores[m,n,b,h] = (sum_i q[m,b,h,i]) * (sum_j k[n,b,h,j]) --
a rank-1 outer product of per-head row-sums. Full Q/K projections are never
needed; only queries @ (per-head-summed Wq) [E,16], computed on host (tiny).

Sharding: 8 cores = 2 (batch) x 4 (head-groups of 4 heads). SPMD program via
run_bass_kernel_spmd; host shards inputs / gathers + reduces outputs.

v4 pipeline per core (batch bi, heads hg*4..hg*4+3), scoresT[n, m] orientation:
  - key-padding mask folded into V: host zeroes masked `values` rows and the
    ones-column comes from an augmented "liveness" feature (K=1 matmul), so
    masked keys contribute exactly 0 and exp needs no bias.
  - off-diag scores on the PE: scT[n,m] = c_n*qs_m + beta_m is rank-2, so one
    K=2 matmul per chunk (lhsT=[c_n;1], rhs=[qs_m;beta_m], bf16) writes the
    score block to PSUM; ACT exps PSUM pairs [128,1024] straight to SBUF bf16.
    beta/c are consistently bf16-rounded on host so the per-row max
    subtraction cancels exactly; per-m scale errors cancel in softmax.
  - diag scores on DVE (needs the per-(n,m) causal tri): stt with
    host-precomputed beta+tri tiles, one wide exp per (t,h).
  - attention (PE, bf16): pooledT[65, m] += [v_h | ones].T @ eT over n-chunks.
  - divide: denominator rows DMA-gathered partition-spread as [128,16] so one
    bf16 vector.reciprocal covers an m-tile, re-broadcast, DVE 2x multiply.
  - out-proj (PE, bf16, K=128): head-pair merge via SBUF DMA, outT accumulated
    over 2 chunks, copied to bf16 (DVE), DMA'd out.
Host assembles: out[m,b,:] = sum_hg outT.T + bo + bv @ Wo.T, with exact numpy
recompute of rows whose live row-max is < -55 (degenerate or needing the
reference's -1000-mask contributions).
"""
import sys

for _p in ("/opt/trn_rl_repo", "/root/.axon_site/_ro/trn_rl_repo"):
    if _p not in sys.path:
        sys.path.append(_p)

import numpy as np
import ml_dtypes

import concourse.bass as bass
import concourse.mybir as mybir
import concourse.tile as tile
from concourse import bacc
from concourse.bass_utils import run_bass_kernel_spmd

# Problem shapes (hardcoded per contract)
M = 2048   # query positions
N = 2048   # key positions
B = 2
E = 1024
H = 16
DH = 64        # head dim
HL = 4         # heads per core
KL = HL * DH   # 256 local pooled dims
KL2 = HL * (DH + 1)  # 260: per head 64 v-dims + ones column
NEG = -1000.0
P = 128
NK = N // P    # 16 n-chunks
T = 4          # m-tiles of 512
MT = 512
NCORES = 8

f32 = mybir.dt.float32
bf16 = mybir.dt.bfloat16

_CACHE = {}


def _build_program():
    if "nc" in _CACHE:
        return _CACHE["nc"]
    nc = bacc.Bacc("TRN2", target_bir_lowering=False, debug=False,
                   num_devices=NCORES)

    vt_d = nc.declare_dram_parameter("vt", [P, 4, (E // P) * MT], bf16, isOutput=False)
    vlive_d = nc.declare_dram_parameter("vlive", [1, N], bf16, isOutput=False)
    wvlt_d = nc.declare_dram_parameter("wvlt", [P, (E // P) * KL2], bf16, isOutput=False)
    wvrow_d = nc.declare_dram_parameter("wvrow", [1, KL2], bf16, isOutput=False)
    wolt_d = nc.declare_dram_parameter("wolt", [P, (KL // P) * E], bf16, isOutput=False)
    qb_d = nc.declare_dram_parameter("qb", [3, HL * M], bf16, isOutput=False)
    cb_d = nc.declare_dram_parameter("cb", [3, NK * HL * P], bf16, isOutput=False)
    zmask_d = nc.declare_dram_parameter("zmask", [P, 2 * P], mybir.dt.uint8, isOutput=False)
    # blocked output: [ob, t, 128, 512] -> host reassembles to [E, M]
    outp_d = nc.declare_dram_parameter("outp", [E // P, T, P, MT], bf16,
                                       isOutput=True)

    with tile.TileContext(nc) as tc:
        with (
            tc.tile_pool(name="const", bufs=1) as const,
            tc.tile_pool(name="scp", bufs=2) as scp,
            tc.tile_pool(name="etp", bufs=8) as etp,
            tc.tile_pool(name="vstream", bufs=2) as vstream,
            tc.tile_pool(name="rspool", bufs=2) as rspool,
            tc.tile_pool(name="ptn", bufs=4) as ptn,
            tc.tile_pool(name="small", bufs=4) as small,
            tc.tile_pool(name="opool", bufs=5) as opool,
            tc.tile_pool(name="dpool", bufs=4, space="DRAM") as dpool,
        ):
            # ---- resident constants (bulk DMAs, few descriptors) ----
            # wvlt split into two TILES so the first v-proj matmuls start
            # as soon as the first half lands (deps are tile-granular)
            wvlt_v = wvlt_d.rearrange("p (ek d) -> p ek d", ek=E // P)
            wvlt_lo = const.tile([P, 4, KL2], bf16)
            nc.sync.dma_start(wvlt_lo[:], wvlt_v[:, 0:4])
            # vt quarter 3 is needed by the very first matmul: issue it 2nd
            vv3 = vt_d[:, 3].rearrange("p (ek n) -> p ek n", ek=E // P)
            vt3_lo = vstream.tile([P, 4, MT], bf16, tag="vt_lo")
            nc.sync.dma_start(vt3_lo[:], vv3[:, 0:4])
            wvlt_hi = const.tile([P, 4, KL2], bf16)
            nc.sync.dma_start(wvlt_hi[:], wvlt_v[:, 4:8])
            vt3_hi = vstream.tile([P, 4, MT], bf16, tag="vt_hi")
            nc.sync.dma_start(vt3_hi[:], vv3[:, 4:8])
            wvrow_sb = const.tile([1, KL2], bf16)
            nc.sync.dma_start(wvrow_sb[:], wvrow_d[:])
            vlive_sb = const.tile([1, N], bf16)
            nc.sync.dma_start(vlive_sb[:], vlive_d[:])
            cb_sb = const.tile([3, NK, HL, P], bf16)
            nc.sync.dma_start(cb_sb[:], cb_d.rearrange("a (k h p) -> a k h p", k=NK, h=HL))
            qb_sb = const.tile([3, HL, M], bf16)
            nc.sync.dma_start(qb_sb[:], qb_d.rearrange("a (h m) -> a h m", h=HL))
            zmask_sb = const.tile([P, 2 * P], mybir.dt.uint8)
            nc.sync.dma_start(zmask_sb[:], zmask_d[:])
            zdata_sb = const.tile([P, 2 * P], bf16)
            nc.vector.memset(zdata_sb[:], 0.0)

            v_sb = const.tile([P, NK, KL2], bf16)

            wolt_sb = const.tile([P, KL // P, E], bf16)
            nc.sync.dma_start(wolt_sb[:], wolt_d.rearrange("p (kb o) -> p kb o", kb=KL // P))

            # ---- stage 2: scores / softmax / attention / out-proj ----
            # Tiles run DESCENDING (t=3 first): t=3 only needs v chunks 12-15
            # (the first v-proj quarter), so stage 2 overlaps stage 1.
            # Each tile's out-projection is DEFERRED and interleaved into the
            # next tile's head loop so the PE stream never stalls on the
            # divide chain (stalls re-throttle the HAM clock gate to 1.2GHz).
            OFF = (0, 128, 384, 768)
            with (
                tc.tile_pool(name="ps_sc", bufs=2, space="PSUM") as ps_sc,
                tc.tile_pool(name="ps_pool", bufs=2, space="PSUM") as ps_pool,
                tc.tile_pool(name="ps_o", bufs=1, space="PSUM") as ps_o,
                tc.tile_pool(name="ps_v", bufs=1, space="PSUM") as ps_v,
            ):
                pending = None      # (pTn2_tile, t) awaiting out-proj

                # ---- v projection, one n-quarter at a time; emitted BETWEEN
                # tiles so its matmuls fill PE gaps and keep the HAM warm.
                # quarter q feeds tile t=q (tile t needs chunks k >= 4t).
                def emit_vproj(q, vt_halves):
                    for nk_r in range(3, -1, -1):
                        k = q * 4 + nk_r
                        vps = ps_v.tile([P, KL2], f32, tag="vps")
                        for ek in range(E // P):
                            vt_h = vt_halves[ek // 4]
                            w_h = wvlt_lo if ek < 4 else wvlt_hi
                            nc.tensor.matmul(
                                vps[:],
                                vt_h[:, ek % 4, nk_r * P:(nk_r + 1) * P],
                                w_h[:, ek % 4, :],
                                start=(ek == 0),
                                stop=False,
                            )
                        # liveness rank-1 update fills the per-head ones cols
                        nc.tensor.matmul(
                            vps[:],
                            vlive_sb[:, k * P:(k + 1) * P],
                            wvrow_sb[:],
                            start=False,
                            stop=True,
                        )
                        nc.any.tensor_copy(out=v_sb[:, k], in_=vps[:])

                def load_vt(q):
                    vv = vt_d[:, q].rearrange("p (ek n) -> p ek n", ek=E // P)
                    lo = vstream.tile([P, 4, MT], bf16, tag="vt_lo")
                    nc.sync.dma_start(lo[:], vv[:, 0:4])
                    hi = vstream.tile([P, 4, MT], bf16, tag="vt_hi")
                    nc.sync.dma_start(hi[:], vv[:, 4:8])
                    return (lo, hi)

                vt_next = (vt3_lo, vt3_hi)

                def emit_outproj(state, obs):
                    pTn2_p, t_p = state
                    for ob in obs:
                        ops = ps_o.tile([P, MT], f32, tag="ops")
                        for kb in range(KL // P):
                            nc.tensor.matmul(
                                ops[:],
                                wolt_sb[:, kb, ob * P:(ob + 1) * P],
                                pTn2_p[:, kb, :],
                                start=(kb == 0),
                                stop=(kb == KL // P - 1),
                            )
                        osb = opool.tile([P, MT], bf16, tag="osb")
                        nc.any.tensor_copy(out=osb[:], in_=ops[:])
                        nc.sync.dma_start(outp_d[ob, t_p], osb[:])

                TORDER = (3, 2, 1, 0)
                for ti, t in enumerate(TORDER):
                    # v-proj for this tile's quarter; prefetch the next one
                    emit_vproj(t, vt_next)
                    if t > 0:
                        vt_next = load_vt(t - 1)
                    pTn2 = ptn.tile([P, KL // P, MT], bf16, tag="ptn2")
                    pool_sbs = []
                    rdall = dpool.tile([HL, MT], bf16, tag="rdall")
                    for h in range(HL):
                        od = 12 - 4 * t
                        et_tiles = []   # (k, et_tile, col_off, W)
                        # --- off-diag scores: K=3 matmuls into PSUM pairs ---
                        for pi in range(od // 2):
                            k0 = 15 - 2 * pi          # pair (k0, k0-1)
                            scps = ps_sc.tile([P, 2 * MT], f32, tag="scps")
                            for j2, k in enumerate((k0, k0 - 1)):
                                nc.tensor.matmul(
                                    scps[:, j2 * MT:(j2 + 1) * MT],
                                    cb_sb[:, k, h, :],
                                    qb_sb[:, h, t * MT:(t + 1) * MT],
                                    start=True,
                                    stop=True,
                                )
                            et = etp.tile([P, 2 * MT], bf16, tag="et")
                            nc.scalar.activation(
                                et[:], scps[:], mybir.ActivationFunctionType.Exp)
                            et_tiles.append((k0, et, 0, MT))
                            et_tiles.append((k0 - 1, et, MT, MT))
                        # --- diagonal: K=3 matmuls too; the causal tri is
                        # applied AFTER exp by predicated-zeroing the last-128
                        # square of each pos block (dead entries can be inf).
                        # pair A: pos3|pos2 (cols 0:512 | 512:896),
                        # pair B: pos1|pos0 (cols 0:256 | 256:384)
                        scpsA = ps_sc.tile([P, 2 * MT], f32, tag="scps")
                        for j2, pos in enumerate((3, 2)):
                            W = (pos + 1) * P
                            nc.tensor.matmul(
                                scpsA[:, j2 * MT:j2 * MT + W],
                                cb_sb[:, 4 * t + pos, h, :],
                                qb_sb[:, h, t * MT:t * MT + W],
                                start=True,
                                stop=True,
                            )
                        etdA = etp.tile([P, 2 * MT], bf16, tag="et")
                        nc.scalar.activation(
                            etdA[:, 0:896], scpsA[:, 0:896],
                            mybir.ActivationFunctionType.Exp)
                        nc.vector.copy_predicated(
                            out=etdA.rearrange("p (a b) -> p a b", b=P)[:, 3:7:3, :],
                            mask=zmask_sb.rearrange("p (a b) -> p a b", b=P),
                            data=zdata_sb.rearrange("p (a b) -> p a b", b=P),
                        )
                        scpsB = ps_sc.tile([P, 2 * MT], f32, tag="scps")
                        for j2, pos in enumerate((1, 0)):
                            W = (pos + 1) * P
                            nc.tensor.matmul(
                                scpsB[:, j2 * P * 2:j2 * P * 2 + W],
                                cb_sb[:, 4 * t + pos, h, :],
                                qb_sb[:, h, t * MT:t * MT + W],
                                start=True,
                                stop=True,
                            )
                        etdB = etp.tile([P, 2 * MT], bf16, tag="et")
                        nc.scalar.activation(
                            etdB[:, 0:384], scpsB[:, 0:384],
                            mybir.ActivationFunctionType.Exp)
                        nc.vector.copy_predicated(
                            out=etdB[:, P:3 * P],
                            mask=zmask_sb[:],
                            data=zdata_sb[:],
                        )
                        et_tiles.append((4 * t + 3, etdA, 0, 512))
                        et_tiles.append((4 * t + 2, etdA, MT, 384))
                        et_tiles.append((4 * t + 1, etdB, 0, 256))
                        et_tiles.append((4 * t + 0, etdB, 2 * P, 128))

                        # --- attention matmuls, k descending ---
                        pool_ps = ps_pool.tile([DH + 1, MT], f32, tag="pool")
                        nmm = len(et_tiles)
                        for i2, (k, et, off, W) in enumerate(et_tiles):
                            nc.tensor.matmul(
                                pool_ps[:, 0:W],
                                v_sb[:, k, h * (DH + 1):(h + 1) * (DH + 1)],
                                et[:, off:off + W],
                                start=(i2 == 0),
                                stop=(i2 == nmm - 1),
                            )
                        # prompt copy frees the PSUM bank + feeds denominators
                        pool_sb = rspool.tile([DH + 1, MT], bf16,
                                              tag=f"pool_sb{h % 2}")
                        nc.any.tensor_copy(out=pool_sb[:], in_=pool_ps[:])
                        pool_sbs.append(pool_sb)
                        nc.sync.dma_start(rdall[h:h + 1, :],
                                          pool_sb[DH:DH + 1, :])
                        # interleave 2 deferred out-proj blocks (prev tile)
                        if pending is not None:
                            emit_outproj(pending, range(2 * h, 2 * h + 2))
                            if h == HL - 1:
                                pending = None
                    # Gather the 4 denominator rows (already in DRAM),
                    # reload partition-spread as [128,16] so ONE reciprocal
                    # covers the m-tile, scatter back, broadcast, multiply.
                    rsg = small.tile([P, HL * MT // P], bf16, tag="rsg")
                    nc.sync.dma_start(
                        rsg[:], rdall.rearrange("a (b x) -> (a b) x", x=HL * MT // P))
                    rsgr = small.tile([P, HL * MT // P], bf16, tag="rsgr")
                    with nc.allow_low_precision(reason="bf16 softmax denominators"):
                        nc.vector.reciprocal(out=rsgr[:], in_=rsg[:])
                    rdall2 = dpool.tile([HL, MT], bf16, tag="rdall2")
                    nc.sync.dma_start(
                        rdall2.rearrange("a (b x) -> (a b) x", x=HL * MT // P), rsgr[:])
                    for h in range(HL):
                        rsb = small.tile([DH, MT], bf16, tag="rsb")
                        nc.sync.dma_start(
                            rsb[:], rdall2[h][None, :].to_broadcast([DH, MT]))
                        if h % 2 == 0:
                            nc.vector.tensor_mul(
                                out=pTn2[0:DH, h // 2, :],
                                in0=pool_sbs[h][0:DH, :],
                                in1=rsb[:],
                            )
                        else:
                            pTn_h = small.tile([DH, MT], bf16, tag="ptnh")
                            nc.vector.tensor_mul(
                                out=pTn_h[:],
                                in0=pool_sbs[h][0:DH, :],
                                in1=rsb[:],
                            )
                            nc.sync.dma_start(pTn2[DH:P, h // 2], pTn_h[:])
                    pending = (pTn2, t)
                # final tile's out-projection (tail)
                emit_outproj(pending, range(E // P))

    nc.compile()
    _CACHE["nc"] = nc
    return nc


def _host_prep(queries, keys, values, Wq, bq, Wk, bk, Wv, bv, Wo, bo, in_mask):
    """Host-side prep. Returns (in_maps, fixup, extras)."""
    qs0 = np.einsum("mbe,he->mbh", queries, Wq.reshape(H, DH, E).sum(1),
                    dtype=np.float32) + bq.reshape(H, DH).sum(1)
    ks = np.einsum("nbe,he->nbh", keys, Wk.reshape(H, DH, E).sum(1),
                   dtype=np.float32) + bk.reshape(H, DH).sum(1)
    # device consumes qs AND c in bf16 (PE rank-2 score build); compute beta
    # from the ROUNDED values so the per-row max subtraction cancels exactly
    qs = qs0.astype(ml_dtypes.bfloat16).astype(np.float32)

    mask3 = in_mask[:, :, None]
    # two-limb bf16 split of the masked key row-sums: c32 = c_hi + c_lo is
    # exactly representable in fp32 and feeds both the PE K=3 score matmul
    # (rows c_hi, c_lo) and the DVE diag stt scalar, so scores carry ~16-bit
    # mantissa precision while staying on the fast bf16 PE path
    cpf = np.where(mask3, 0.0, ks).astype(np.float32)
    c_hi = cpf.astype(ml_dtypes.bfloat16)
    c_lo = (cpf - c_hi.astype(np.float32)).astype(ml_dtypes.bfloat16)
    cp = c_hi.astype(np.float32) + c_lo.astype(np.float32)

    cmax = np.where(mask3, -np.inf, cp)
    cmax = np.maximum.accumulate(cmax[::-1], axis=0)[::-1]    # suffix max, n>=m
    cmin = np.where(mask3, np.inf, cp)
    cmin = np.minimum.accumulate(cmin[::-1], axis=0)[::-1]
    nonempty = np.maximum.accumulate((~in_mask)[::-1], axis=0)[::-1]  # [n, b]

    with np.errstate(invalid="ignore"):
        A = np.where(qs >= 0, qs * cmax, qs * cmin)           # [m, b, H]
    A = np.where(nonempty[:, :, None], A, -np.inf)
    # beta is clamped at +60 so masked entries (c=0 -> et=exp(beta)) stay
    # finite; any per-row shift cancels in softmax. Rows where the live max
    # is < -55 lose exact stabilization (or need the reference's -1000-mask
    # contributions) and are recomputed exactly on host.
    fixup_rows = np.any(~(A > -55.0), axis=2)                 # [m, b] (nan-safe)
    with np.errstate(invalid="ignore"):
        beta = np.where(np.isfinite(A), np.minimum(-A, 60.0), 60.0)
    beta = np.where(np.isnan(beta), 60.0, beta).astype(np.float32)

    # fold the key-padding mask into V: masked rows contribute 0
    values_m = np.where(mask3, 0.0, values).astype(np.float32)
    live = (~in_mask).astype(np.float32)                      # [n, b]

    in_maps = []
    def pmajor(a, p=P):
        """[X*p, Y] -> [p, X*Y]: partition-major packing for 1-run-per-
        partition DMA loads matching 'p (x y) -> p x y' device views."""
        X = a.shape[0] // p
        return np.ascontiguousarray(
            a.reshape(X, p, a.shape[1]).transpose(1, 0, 2).reshape(p, -1))

    def pack_vt(vT):
        # [E, N] -> [P, 4, (E//P)*MT]: quarter-major, then ek-major
        a = vT.reshape(E // P, P, 4, MT)          # [ek, p, q, mt]
        return np.ascontiguousarray(
            a.transpose(1, 2, 0, 3).reshape(P, 4, (E // P) * MT))

    vt_by_b = [pack_vt(values_m[:, bi, :].T.astype(ml_dtypes.bfloat16))
               for bi in range(B)]
    # zmask: per-square causal dead mask (1 = dead), duplicated [sq|sq]
    sq = (np.arange(P)[:, None] < np.arange(P)[None, :]).astype(np.float32)
    zmask = np.concatenate([sq, sq], axis=1)

    # wvrow: ones at the per-head ones-column positions
    wvrow = np.zeros((1, KL2), np.float32)
    for h in range(HL):
        wvrow[0, h * (DH + 1) + DH] = 1.0

    for c in range(NCORES):
        bi, hg = c // 4, c % 4
        lh = slice(hg * HL, (hg + 1) * HL)
        ds = slice(hg * KL, (hg + 1) * KL)
        # augmented WvL^T: per head 64 weight cols then a zero col
        wvl = Wv[ds, :].reshape(HL, DH, E)
        wvl_aug = np.zeros((E, KL2), np.float32)
        for h in range(HL):
            wvl_aug[:, h * (DH + 1):h * (DH + 1) + DH] = wvl[h].T
        # qb: rows (qs, qs, beta); cb: rows (c_hi, c_lo, 1)
        qsl_flat = np.ascontiguousarray(qs[:, bi, lh].T).reshape(-1)
        qb = np.stack([qsl_flat, qsl_flat,
                       np.ascontiguousarray(beta[:, bi, lh].T).reshape(-1)])
        def _cl(a):
            return np.ascontiguousarray(
                np.asarray(a, np.float32)[:, bi, lh]
                .reshape(NK, P, HL).transpose(0, 2, 1)).reshape(-1)
        cb = np.stack([_cl(c_hi), _cl(c_lo), np.ones(NK * HL * P, np.float32)])
        in_maps.append({
            "vt": vt_by_b[bi],
            "vlive": np.ascontiguousarray(
                live[:, bi][None, :]).astype(ml_dtypes.bfloat16),
            "wvlt": pmajor(wvl_aug.astype(ml_dtypes.bfloat16)),
            "wvrow": wvrow.astype(ml_dtypes.bfloat16),
            "wolt": pmajor(Wo[:, ds].T.astype(ml_dtypes.bfloat16)),
            "qb": qb.astype(ml_dtypes.bfloat16),
            "cb": cb.astype(ml_dtypes.bfloat16),
            "zmask": zmask.astype(np.uint8),
        })
    return in_maps, fixup_rows, (qs0, ks)


def _fixup_row(out, m, bi, qs, ks, values, Wv, bv, Wo, bo, in_mask):
    """Exact numpy recompute of one output row (degenerate / extreme rows)."""
    pot = qs[m, bi, :][None, :] * ks[:, bi, :]                # [n, H]
    pot = np.where(in_mask[:, bi][:, None], NEG, pot)
    causal = np.arange(N) < m                                 # mask n < m
    pot = np.where(causal[:, None], NEG, pot)
    pot = pot - pot.max(axis=0, keepdims=True)
    w = np.exp(pot)
    w = w / w.sum(axis=0, keepdims=True)                      # [n, H]
    v = (values[:, bi, :] @ Wv.T + bv).reshape(N, H, DH)
    pooled = np.einsum("nh,nhd->hd", w, v).reshape(E)
    out[m, bi, :] = pooled @ Wo.T + bo


def kernel(queries, keys, values, Wq, bq, Wk, bk, Wv, bv, Wo, bo, in_mask,
           _trace=False):
    args = (queries, keys, values, Wq, bq, Wk, bk, Wv, bv, Wo, bo)
    args = tuple(np.asarray(a, np.float32) for a in args)
    in_mask = np.asarray(in_mask, bool)
    (queries, keys, values, Wq, bq, Wk, bk, Wv, bv, Wo, bo) = args

    nc = _build_program()
    in_maps, fixup_rows, (qs, ks) = _host_prep(
        queries, keys, values, Wq, bq, Wk, bk, Wv, bv, Wo, bo, in_mask)

    res = run_bass_kernel_spmd(nc, in_maps, list(range(NCORES)), trace=_trace)
    results = res.results

    out = np.zeros((M, B, E), np.float32)
    for c in range(NCORES):
        bi = c // 4
        blk = np.asarray(results[c]["outp"], np.float32)   # [8, 4, 128, 512]
        outT = blk.transpose(0, 2, 1, 3).reshape(E, M)
        out[:, bi, :] += outT.T
    out += (bo + bv @ Wo.T)[None, None, :]

    for m, bi in zip(*np.nonzero(fixup_rows)):
        _fixup_row(out, m, bi, qs, ks, values, Wv, bv, Wo, bo, in_mask)

    if _trace:
        return out, res
    return out
